# revision 4
# baseline (speedup 1.0000x reference)
"""H2GCNConv on 8 Trainium2 NeuronCores.

out = concat([A1 @ x, A2 @ x], axis=1) where A_h is sparse [N, N] given as
(row=dest, col=src, val) edge lists.

Strategy (dest-sharded SpMM via fp8 gather + segment-matmul):
  - Destination rows are partitioned across 8 cores (6250 rows each),
    then into 128-row dest tiles (49 per core).
  - x is cast to fp8 e3m4 (rel err ~1.4e-2 on this graph, under the 2e-2
    gate) and replicated; each core gathers its edges' source rows (256B
    each) from HBM with SWDGE dma_gather.
  - Edges are grouped by (dest tile, column half) — the half split keeps
    gather indices within int16 — with hop-1 edges first, then hop-2,
    each sorted by source for HBM page locality, padded to a shared
    cross-core layout.  One gather per (tile, half).
  - For each 128-edge chunk, a selection matrix S[e, d] = val[e] *
    (d == dest_local[e]) is built with a single fused DVE tensor_scalar
    (iota == dest) * val using per-partition f32 scalar tables, then the
    tensor engine computes psum[d, :] += S.T @ msgs (fp16 x fp8 -> fp32
    PSUM), performing scale + segment-sum in one matmul.  A chunk that
    straddles the hop boundary is issued to both hops' accumulations
    with the other hop's val entries zeroed.
  - PSUM accumulates per (tile, hop); results are staged to one
    [128, 512] SBUF tile per dest tile and written with a single DMA.
"""

import sys

if "/opt/trn_rl_repo" not in sys.path:
    sys.path.insert(0, "/opt/trn_rl_repo")

import numpy as np
import ml_dtypes

P = 128


def _host_build(x, row1, col1, val1, row2, col2, val2, ncores):
    n_nodes, d_feat = x.shape
    rpc = n_nodes // ncores
    T = -(-rpc // P)
    split = -(-n_nodes // 2)
    # keep both halves within int16 gather-index range
    assert split <= 32767 and n_nodes - split <= 32767

    # combined edge list over both hops
    rows = np.concatenate([np.asarray(row1), np.asarray(row2)])
    cols = np.concatenate([np.asarray(col1), np.asarray(col2)])
    vals = np.concatenate([np.asarray(val1), np.asarray(val2)]).astype(np.float32)
    hop = np.concatenate([
        np.zeros(len(row1), np.int64), np.ones(len(row2), np.int64)])

    core = rows // rpc
    local = rows - core * rpc
    t = local >> 7
    ld = (local & (P - 1)).astype(np.float32)
    half = (cols >= split).astype(np.int64)
    idx = (cols - half * split).astype(np.int16)

    # sort by (core, t, half, hop, col): sections are (core, t, half) with
    # hop-1 edges first then hop-2, each in ascending source order
    order = np.lexsort((cols, hop, half, t, core))
    skey = ((core * T + t) * 2 + half)[order]
    hop_s = hop[order]

    # per-(core, t, half, hop) counts
    key4 = (skey * 2 + hop_s)
    counts = np.bincount(key4, minlength=ncores * T * 2 * 2).reshape(
        ncores, T, 2, 2)
    nmax = counts.max(axis=0)  # [T, 2, 2] max over cores per (t, half, hop)
    n1m = nmax[:, :, 0]
    n2m = nmax[:, :, 1]
    S = -(-(n1m + n2m) // P)  # chunks per (t, half)
    S = np.maximum(S, 1)

    # device edge-space layout: for t: [half0 | half1], each S[t,half]*P slots
    sec_off = np.zeros((T, 2), dtype=np.int64)  # start slot of (t, half)
    tile_chunks = S.sum(axis=1)
    tile_off_chunks = np.concatenate([[0], np.cumsum(tile_chunks)])
    for tt in range(T):
        sec_off[tt, 0] = tile_off_chunks[tt] * P
        sec_off[tt, 1] = tile_off_chunks[tt] * P + S[tt, 0] * P
    tot_chunks = int(tile_off_chunks[-1])
    pad_e = tot_chunks * P

    # rank of each edge within its (core, t, half, hop) bucket
    cs = np.concatenate([[0], np.cumsum(counts.reshape(-1))])
    rank = np.arange(len(rows)) - cs[key4]

    core_s = skey // (T * 2)
    t_s = (skey // 2) % T
    half_s = skey % 2
    hop_off = np.where(hop_s == 0, 0, n1m[t_s, half_s])
    pos = sec_off[t_s, half_s] + hop_off + rank

    pad_idx = np.zeros((ncores, pad_e), dtype=np.int16)
    pad_ld = np.zeros((ncores, pad_e), dtype=np.float32)
    pad_val = np.zeros((ncores, pad_e), dtype=np.float32)
    pad_hop = np.full((ncores, pad_e), -1, dtype=np.int8)  # -1 = padding
    pad_idx[core_s, pos] = idx[order]
    pad_ld[core_s, pos] = ld[order]
    pad_val[core_s, pos] = vals[order]
    pad_hop[core_s, pos] = hop_s.astype(np.int8)

    # matmul step lists: per (t, hop) the (table_col, msgs_chunk) pairs.
    # hop1 uses chunks [0, ceil(n1m/P)) of each half; hop2 uses
    # [n1m//P, S).  A chunk straddling the boundary appears in both.
    steps = [[[], []] for _ in range(T)]  # steps[t][h] = list of chunk info
    step_src = []  # (t, half, chunk, hop) per table column
    for tt in range(T):
        for h in range(2):
            for hf in range(2):
                a = int(n1m[tt, hf])
                if h == 0:
                    lo, hi = 0, -(-a // P)
                else:
                    lo, hi = a // P, int(S[tt, hf])
                base_chunk = int(S[tt, 0]) if hf else 0
                for c in range(lo, hi):
                    col_id = len(step_src)
                    step_src.append((tt, hf, c, h))
                    steps[tt][h].append((col_id, base_chunk + c))
    n_steps = len(step_src)

    # per-step dest/val tables [ncores, 128, n_steps] f32, hop-masked
    dest_tab = np.zeros((ncores, P, n_steps), dtype=np.float32)
    val_tab = np.zeros((ncores, P, n_steps), dtype=np.float32)
    for col_id, (tt, hf, c, h) in enumerate(step_src):
        s0 = int(sec_off[tt, hf]) + c * P
        seg_ld = pad_ld[:, s0 : s0 + P]
        seg_val = pad_val[:, s0 : s0 + P]
        seg_hop = pad_hop[:, s0 : s0 + P]
        m = seg_hop == h
        dest_tab[:, :, col_id] = np.where(m, seg_ld, 0.0)
        val_tab[:, :, col_id] = np.where(m, seg_val, 0.0)

    # idx dram layout: per (t, half) section, [16, n/16] wrap replicated to
    # 128 partitions
    idx_cols = pad_e // 16
    idx_arr = np.zeros((ncores, 16, idx_cols), dtype=np.int16)
    for tt in range(T):
        for hf in range(2):
            o = int(sec_off[tt, hf])
            n = int(S[tt, hf]) * P
            seg = pad_idx[:, o : o + n].reshape(ncores, n // 16, 16)
            idx_arr[:, :, o // 16 : (o + n) // 16] = seg.transpose(0, 2, 1)
    idx_arr = np.tile(idx_arr, (1, 8, 1))  # [ncores, 128, idx_cols]

    iota = np.broadcast_to(
        np.arange(P, dtype=np.float16)[None, :], (P, P)
    ).copy()

    x8 = np.asarray(x).astype(ml_dtypes.float8_e3m4)

    meta = dict(
        ncores=ncores, rpc=rpc, T=T, split=split, n_nodes=n_nodes,
        d_feat=d_feat, S=S, tile_off_chunks=tile_off_chunks,
        tot_chunks=tot_chunks, sec_off=sec_off, steps=steps,
        n_steps=n_steps, idx_cols=idx_cols,
    )
    per_core = [
        dict(
            x8=x8,
            idx=np.ascontiguousarray(idx_arr[c]),
            dest=np.ascontiguousarray(dest_tab[c]),
            val=np.ascontiguousarray(val_tab[c]),
            iota=iota,
        )
        for c in range(ncores)
    ]
    return meta, per_core


def _build_program(meta, repeat=1):
    from concourse import bacc, mybir, tile

    T = meta["T"]
    rpc = meta["rpc"]
    split = meta["split"]
    n_nodes = meta["n_nodes"]
    d = meta["d_feat"]
    S = meta["S"]
    sec_off = meta["sec_off"]
    steps = meta["steps"]
    n_steps = meta["n_steps"]
    tot_chunks = meta["tot_chunks"]
    idx_cols = meta["idx_cols"]

    nc = bacc.Bacc("TRN2", target_bir_lowering=False, debug=False,
                   num_devices=meta["ncores"])

    fp8 = mybir.dt.float8e3
    fp16 = mybir.dt.float16
    f32 = mybir.dt.float32
    eq = mybir.AluOpType.is_equal
    mult = mybir.AluOpType.mult

    x8 = nc.dram_tensor("x8", [n_nodes, d], fp8, kind="ExternalInput")
    idx_d = nc.dram_tensor("idx", [P, idx_cols], mybir.dt.int16,
                           kind="ExternalInput")
    dest_d = nc.dram_tensor("dest", [P, n_steps], f32, kind="ExternalInput")
    val_d = nc.dram_tensor("val", [P, n_steps], f32, kind="ExternalInput")
    iota_d = nc.dram_tensor("iota", [P, P], fp16, kind="ExternalInput")
    out_d = nc.dram_tensor("out", [rpc, 2 * d], f32, kind="ExternalOutput")

    with tile.TileContext(nc) as tc:
        with (
            tc.tile_pool(name="const", bufs=1) as constp,
            tc.tile_pool(name="idx", bufs=4) as idxp,
            tc.tile_pool(name="msgs", bufs=3) as msgsp,
            tc.tile_pool(name="sel", bufs=3) as selp,
            tc.tile_pool(name="psum", bufs=4, space="PSUM") as psump,
            tc.tile_pool(name="stage", bufs=4) as stagep,
        ):
            iota_sb = constp.tile([P, P], fp16, tag="iota")
            nc.sync.dma_start(iota_sb[:, :], iota_d[:, :])
            dest_sb = constp.tile([P, n_steps], f32, tag="dest")
            nc.sync.dma_start(dest_sb[:, :], dest_d[:, :])
            val_sb = constp.tile([P, n_steps], f32, tag="val")
            nc.sync.dma_start(val_sb[:, :], val_d[:, :])

            for rep in range(repeat):
                for t in range(T):
                    rows = min(P, rpc - t * P)
                    s_lo, s_hi = int(S[t, 0]), int(S[t, 1])
                    msgs = msgsp.tile([P, s_lo + s_hi, d], fp8, tag="msgs")
                    for hf in range(2):
                        n = (s_hi if hf else s_lo) * P
                        o = int(sec_off[t, hf])
                        it = idxp.tile([P, n // 16], mybir.dt.int16, tag="idx")
                        nc.sync.dma_start(
                            it[:, :], idx_d[:, o // 16 : (o + n) // 16]
                        )
                        src = x8[0:split, :] if hf == 0 else x8[split:n_nodes, :]
                        coff = s_lo if hf else 0
                        nc.gpsimd.dma_gather(
                            msgs[:, coff : coff + n // P, :],
                            src,
                            it[:, :],
                            n,
                            n,
                            d,
                            single_packet=False,
                        )
                    stage = stagep.tile([P, 2 * d], f32, tag="st")
                    for h in range(2):
                        sl = steps[t][h]
                        sel = selp.tile([P, len(sl), P], fp16, tag="sel")
                        for j, (sc, mc) in enumerate(sl):
                            nc.vector.tensor_scalar(
                                sel[:, j, :],
                                iota_sb[:, :],
                                dest_sb[:, sc : sc + 1],
                                val_sb[:, sc : sc + 1],
                                eq,
                                mult,
                            )
                        ps = psump.tile([P, d], f32, tag="ps")
                        for j, (sc, mc) in enumerate(sl):
                            nc.tensor.matmul(
                                ps[:, :],
                                sel[:, j, :],
                                msgs[:, mc, :],
                                start=(j == 0),
                                stop=(j == len(sl) - 1),
                            )
                        nc.scalar.copy(stage[:, h * d : (h + 1) * d], ps[:, :])
                    nc.sync.dma_start(
                        out_d[t * P : t * P + rows, :], stage[:rows, :]
                    )
    nc.compile()
    return nc


def kernel(x, row1, col1, val1, row2, col2, val2):
    from concourse.bass_utils import run_bass_kernel_spmd

    ncores = 8
    meta, per_core = _host_build(x, row1, col1, val1, row2, col2, val2, ncores)
    nc = _build_program(meta)
    res = run_bass_kernel_spmd(nc, per_core, list(range(ncores)))
    rpc = meta["rpc"]
    d = meta["d_feat"]
    out = np.empty((x.shape[0], 2 * d), dtype=np.float32)
    for c in range(ncores):
        out[c * rpc : (c + 1) * rpc] = res.results[c]["out"]
    return out


# revision 23
# speedup vs baseline: 1.0781x; 1.0781x over previous
"""H2GCNConv on 8 Trainium2 NeuronCores.

out = concat([A1 @ x, A2 @ x], axis=1) where A_h is sparse [N, N] given as
(row=dest, col=src, val) edge lists.

Strategy (dest-sharded SpMM via fp16 gather + segment-matmul):
  - Destination rows are partitioned across 8 cores (6250 rows each),
    then into 128-row dest tiles (49 per core).
  - x is cast to fp16 and replicated; each core gathers its edges'
    source rows (512B each) from HBM with SWDGE dma_gather.  512B
    descriptors are essential: sub-512B descriptors run ~7x slower on
    real HW (read-modify-write path), which is why x stays fp16 rather
    than fp8.  Each section's gather is split across all 4 SWDGE queues
    — descriptor generation/drain parallelizes ~4x across queue
    contexts (946us -> 243us for the gather alone).
  - Edges are grouped by (dest tile, column half) — the half split keeps
    gather indices within int16 — with hop-1 edges first, then hop-2,
    each sorted by source for HBM page locality, padded to a shared
    cross-core layout.
  - For each 128-edge chunk, a selection matrix S[e, d] = val[e] *
    (d == dest_local[e]) is built with a single fused DVE tensor_scalar
    (iota == dest) * val using per-partition f32 scalar tables (one 4x-
    mode op per chunk; tensor_tensor with broadcast operands drops the
    DVE to 1x mode and was the original bottleneck).  The tensor engine
    then computes psum[d, :] += S.T @ msgs (fp16 x fp16 -> fp32 PSUM),
    performing scale + segment-sum in one matmul.  A chunk that
    straddles the hop boundary is issued to both hops' accumulations
    with the other hop's val entries zeroed.
  - PSUM accumulates per (tile, hop); results are staged to one
    [128, 512] fp16 SBUF tile per dest tile, written with a single DMA,
    and upcast to f32 on the host.
"""

import sys

if "/opt/trn_rl_repo" not in sys.path:
    sys.path.insert(0, "/opt/trn_rl_repo")

import numpy as np
import ml_dtypes

P = 128


def _host_build(x, row1, col1, val1, row2, col2, val2, ncores, xdtype="fp16"):
    n_nodes, d_feat = x.shape
    rpc = n_nodes // ncores
    T = -(-rpc // P)
    split = -(-n_nodes // 2)
    # keep both halves within int16 gather-index range
    assert split <= 32767 and n_nodes - split <= 32767

    # combined edge list over both hops
    rows = np.concatenate([np.asarray(row1), np.asarray(row2)])
    cols = np.concatenate([np.asarray(col1), np.asarray(col2)])
    vals = np.concatenate([np.asarray(val1), np.asarray(val2)]).astype(np.float32)
    hop = np.concatenate([
        np.zeros(len(row1), np.int64), np.ones(len(row2), np.int64)])

    core = rows // rpc
    local = rows - core * rpc
    t = local >> 7
    ld = (local & (P - 1)).astype(np.float32)
    half = (cols >= split).astype(np.int64)
    idx = (cols - half * split).astype(np.int16)

    # sort by (core, t, half, hop, col): sections are (core, t, half) with
    # hop-1 edges first then hop-2, each in ascending source order
    order = np.lexsort((cols, hop, half, t, core))
    skey = ((core * T + t) * 2 + half)[order]
    hop_s = hop[order]

    # per-(core, t, half, hop) counts
    key4 = (skey * 2 + hop_s)
    counts = np.bincount(key4, minlength=ncores * T * 2 * 2).reshape(
        ncores, T, 2, 2)
    nmax = counts.max(axis=0)  # [T, 2, 2] max over cores per (t, half, hop)
    n1m = nmax[:, :, 0]
    n2m = nmax[:, :, 1]
    S = -(-(n1m + n2m) // P)  # chunks per (t, half)
    S = np.maximum(S, 1)

    # device edge-space layout: for t: [half0 | half1], each S[t,half]*P slots
    sec_off = np.zeros((T, 2), dtype=np.int64)  # start slot of (t, half)
    tile_chunks = S.sum(axis=1)
    tile_off_chunks = np.concatenate([[0], np.cumsum(tile_chunks)])
    for tt in range(T):
        sec_off[tt, 0] = tile_off_chunks[tt] * P
        sec_off[tt, 1] = tile_off_chunks[tt] * P + S[tt, 0] * P
    tot_chunks = int(tile_off_chunks[-1])
    pad_e = tot_chunks * P

    # rank of each edge within its (core, t, half, hop) bucket
    cs = np.concatenate([[0], np.cumsum(counts.reshape(-1))])
    rank = np.arange(len(rows)) - cs[key4]

    core_s = skey // (T * 2)
    t_s = (skey // 2) % T
    half_s = skey % 2
    hop_off = np.where(hop_s == 0, 0, n1m[t_s, half_s])
    pos = sec_off[t_s, half_s] + hop_off + rank

    pad_idx = np.zeros((ncores, pad_e), dtype=np.int16)
    pad_ld = np.zeros((ncores, pad_e), dtype=np.float32)
    pad_val = np.zeros((ncores, pad_e), dtype=np.float32)
    pad_hop = np.full((ncores, pad_e), -1, dtype=np.int8)  # -1 = padding
    pad_idx[core_s, pos] = idx[order]
    pad_ld[core_s, pos] = ld[order]
    pad_val[core_s, pos] = vals[order]
    pad_hop[core_s, pos] = hop_s.astype(np.int8)

    # matmul step lists: per (t, hop) the (table_col, msgs_chunk) pairs.
    # hop1 uses chunks [0, ceil(n1m/P)) of each half; hop2 uses
    # [n1m//P, S).  A chunk straddling the boundary appears in both.
    steps = [[[], []] for _ in range(T)]  # steps[t][h] = list of chunk info
    step_src = []  # (t, half, chunk, hop) per table column
    for tt in range(T):
        for h in range(2):
            for hf in range(2):
                a = int(n1m[tt, hf])
                if h == 0:
                    lo, hi = 0, -(-a // P)
                else:
                    lo, hi = a // P, int(S[tt, hf])
                base_chunk = int(S[tt, 0]) if hf else 0
                for c in range(lo, hi):
                    col_id = len(step_src)
                    step_src.append((tt, hf, c, h))
                    steps[tt][h].append((col_id, base_chunk + c))
    n_steps = len(step_src)

    # per-step dest/val tables [ncores, 128, n_steps] f32, hop-masked
    dest_tab = np.zeros((ncores, P, n_steps), dtype=np.float32)
    val_tab = np.zeros((ncores, P, n_steps), dtype=np.float32)
    for col_id, (tt, hf, c, h) in enumerate(step_src):
        s0 = int(sec_off[tt, hf]) + c * P
        seg_ld = pad_ld[:, s0 : s0 + P]
        seg_val = pad_val[:, s0 : s0 + P]
        seg_hop = pad_hop[:, s0 : s0 + P]
        m = seg_hop == h
        dest_tab[:, :, col_id] = np.where(m, seg_ld, 0.0)
        val_tab[:, :, col_id] = np.where(m, seg_val, 0.0)

    # idx dram layout: per (t, half) section, [16, n/16] wrap replicated to
    # 128 partitions
    idx_cols = pad_e // 16
    idx_arr = np.zeros((ncores, 16, idx_cols), dtype=np.int16)
    for tt in range(T):
        for hf in range(2):
            o = int(sec_off[tt, hf])
            n = int(S[tt, hf]) * P
            seg = pad_idx[:, o : o + n].reshape(ncores, n // 16, 16)
            idx_arr[:, :, o // 16 : (o + n) // 16] = seg.transpose(0, 2, 1)
    idx_arr = np.tile(idx_arr, (1, 8, 1))  # [ncores, 128, idx_cols]

    iota = np.broadcast_to(
        np.arange(P, dtype=np.float16)[None, :], (P, P)
    ).copy()

    x8 = np.asarray(x).astype(
        ml_dtypes.float8_e3m4 if xdtype == "fp8" else np.float16)

    meta = dict(
        ncores=ncores, rpc=rpc, T=T, split=split, n_nodes=n_nodes,
        d_feat=d_feat, S=S, tile_off_chunks=tile_off_chunks,
        tot_chunks=tot_chunks, sec_off=sec_off, steps=steps,
        n_steps=n_steps, idx_cols=idx_cols, xdtype=xdtype,
    )
    per_core = [
        dict(
            x8=x8,
            idx=np.ascontiguousarray(idx_arr[c]),
            dest=np.ascontiguousarray(dest_tab[c]),
            val=np.ascontiguousarray(val_tab[c]),
            iota=iota,
        )
        for c in range(ncores)
    ]
    return meta, per_core


def _build_program(meta, repeat=1, out_fp16=True, msgs_bufs=5, sel_bufs=3):
    from concourse import bacc, mybir, tile

    T = meta["T"]
    rpc = meta["rpc"]
    split = meta["split"]
    n_nodes = meta["n_nodes"]
    d = meta["d_feat"]
    S = meta["S"]
    sec_off = meta["sec_off"]
    steps = meta["steps"]
    n_steps = meta["n_steps"]
    tot_chunks = meta["tot_chunks"]
    idx_cols = meta["idx_cols"]

    nc = bacc.Bacc("TRN2", target_bir_lowering=False, debug=False,
                   num_devices=meta["ncores"], num_swdge_queues=4)

    fp8 = (mybir.dt.float8e3 if meta.get("xdtype", "fp16") == "fp8"
           else mybir.dt.float16)
    fp16 = mybir.dt.float16
    f32 = mybir.dt.float32
    eq = mybir.AluOpType.is_equal
    mult = mybir.AluOpType.mult

    x8 = nc.dram_tensor("x8", [n_nodes, d], fp8, kind="ExternalInput")
    idx_d = nc.dram_tensor("idx", [P, idx_cols], mybir.dt.int16,
                           kind="ExternalInput")
    dest_d = nc.dram_tensor("dest", [P, n_steps], f32, kind="ExternalInput")
    val_d = nc.dram_tensor("val", [P, n_steps], f32, kind="ExternalInput")
    iota_d = nc.dram_tensor("iota", [P, P], fp16, kind="ExternalInput")
    out_dt = fp16 if out_fp16 else f32
    out_d = nc.dram_tensor("out", [rpc, 2 * d], out_dt, kind="ExternalOutput")

    with tile.TileContext(nc) as tc:
        with (
            tc.tile_pool(name="const", bufs=1) as constp,
            tc.tile_pool(name="idx", bufs=8) as idxp,
            tc.tile_pool(name="msgs", bufs=msgs_bufs) as msgsp,
            tc.tile_pool(name="sel", bufs=sel_bufs) as selp,
            tc.tile_pool(name="psum", bufs=6, space="PSUM") as psump,
            tc.tile_pool(name="stage", bufs=4) as stagep,
        ):
            iota_sb = constp.tile([P, P], fp16, tag="iota")
            nc.sync.dma_start(iota_sb[:, :], iota_d[:, :])
            dest_sb = constp.tile([P, n_steps], f32, tag="dest")
            nc.sync.dma_start(dest_sb[:, :], dest_d[:, :])
            val_sb = constp.tile([P, n_steps], f32, tag="val")
            nc.sync.dma_start(val_sb[:, :], val_d[:, :])

            for rep in range(repeat):
                for t in range(T):
                    rows = min(P, rpc - t * P)
                    s_lo, s_hi = int(S[t, 0]), int(S[t, 1])
                    msgs = msgsp.tile([P, s_lo + s_hi, d], fp8, tag="msgs")
                    for hf in range(2):
                        n = (s_hi if hf else s_lo) * P
                        o = int(sec_off[t, hf])
                        it = idxp.tile([P, n // 16], mybir.dt.int16, tag="idx")
                        nc.sync.dma_start(
                            it[:, :], idx_d[:, o // 16 : (o + n) // 16]
                        )
                        src = x8[0:split, :] if hf == 0 else x8[split:n_nodes, :]
                        coff = s_lo if hf else 0
                        # split each section's gather across the 4 SWDGE
                        # queues (~4x faster descriptor generation/drain)
                        nchunk = n // P
                        per_q = -(-nchunk // 4)
                        for q in range(4):
                            c0 = q * per_q
                            c1 = min(nchunk, c0 + per_q)
                            if c0 >= c1:
                                continue
                            nn = (c1 - c0) * P
                            nc.gpsimd.dma_gather(
                                msgs[:, coff + c0 : coff + c1, :],
                                src,
                                it[:, c0 * 8 : c0 * 8 + nn // 16],
                                nn,
                                nn,
                                d,
                                single_packet=False,
                                queue_num=q,
                            )
                    stage = stagep.tile([P, 2 * d], out_dt, tag="st")
                    for h in range(2):
                        sl = steps[t][h]
                        sel = selp.tile([P, len(sl), P], fp16, tag="sel")
                        for j, (sc, mc) in enumerate(sl):
                            nc.vector.tensor_scalar(
                                sel[:, j, :],
                                iota_sb[:, :],
                                dest_sb[:, sc : sc + 1],
                                val_sb[:, sc : sc + 1],
                                eq,
                                mult,
                            )
                        ps = psump.tile([P, d], f32, tag="ps")
                        for j, (sc, mc) in enumerate(sl):
                            nc.tensor.matmul(
                                ps[:, :],
                                sel[:, j, :],
                                msgs[:, mc, :],
                                start=(j == 0),
                                stop=(j == len(sl) - 1),
                            )
                        nc.scalar.copy(stage[:, h * d : (h + 1) * d], ps[:, :])
                    nc.sync.dma_start(
                        out_d[t * P : t * P + rows, :], stage[:rows, :]
                    )
    nc.compile()
    return nc


def kernel(x, row1, col1, val1, row2, col2, val2):
    from concourse.bass_utils import run_bass_kernel_spmd

    ncores = 8
    meta, per_core = _host_build(x, row1, col1, val1, row2, col2, val2, ncores)
    nc = _build_program(meta)
    res = run_bass_kernel_spmd(nc, per_core, list(range(ncores)))
    rpc = meta["rpc"]
    d = meta["d_feat"]
    out = np.empty((x.shape[0], 2 * d), dtype=np.float32)
    for c in range(ncores):
        out[c * rpc : (c + 1) * rpc] = res.results[c]["out"].astype(np.float32)
    return out
